# revision 1
# baseline (speedup 1.0000x reference)
"""Masked multi-head attention on 8 NeuronCores (faithful torch raw-view semantics).

The reference reshapes (bs, sql, nh*edim) -> (bs, nh, sql, edim) as a RAW VIEW:
head h's length-1024 pseudo-sequence is built from x rows 128h..128h+127 (each
row contributes 8 pseudo-positions, one per 256-col block of the projection),
and output rows 128h..128h+128 depend only on head h. So the work splits into
32 independent (batch, head) pairs -> 4 per core, no cross-core reduction.

Per (b, h): Q/K/V = x[b,128h:128h+128] @ w{q,k,v}.T + b (full 2048-wide), viewed
as (1024, 256) row-major. We index pseudo-positions in PERMUTED order
u = cb*128 + r (s' = r*8 + cb), consistently on both attention axes, which makes
every layout a contiguous matmul tile. The causal mask (on s') is precomputed
host-side for this ordering (16 bf16 tiles, shared by all heads/cores).

Transposed-score formulation: S^T = K^T.T @ Q^T with d-major Q^T/K^T straight
from the projection matmuls; softmax denominator = ones-column matmul; P^T feeds
P@V directly. No on-chip transposes anywhere. Q weights/bias pre-scaled by 1/16.
"""

import sys

sys.path.insert(0, "/opt/trn_rl_repo")

import ml_dtypes
import numpy as np

from concourse import bacc, mybir
from concourse.tile import TileContext
from concourse.bass_utils import run_bass_kernel_spmd

EDIM = 256
BS = 4
SQL = 1024
HPC = 4           # heads per core
NCORES = 8
FDT = mybir.dt.float32
RDT = mybir.dt.float32r    # matmul-feeding storage: 4x faster PE, rounded fp32
NEG = -1.0e30

_cache = {}


def _build():
    nc = bacc.Bacc(dynamic_dma_scratch_size=512)

    xt0 = nc.declare_dram_parameter("xt0", [128, 512], RDT, isOutput=False)
    xt1 = nc.declare_dram_parameter("xt1", [128, 512], RDT, isOutput=False)
    wqk0 = nc.declare_dram_parameter("wqk0", [128, 4096], RDT, isOutput=False)
    wqk1 = nc.declare_dram_parameter("wqk1", [128, 4096], RDT, isOutput=False)
    wv0 = nc.declare_dram_parameter("wv0", [128, 2048], RDT, isOutput=False)
    wv1 = nc.declare_dram_parameter("wv1", [128, 2048], RDT, isOutput=False)
    bqk = nc.declare_dram_parameter("bqk", [128, 32], FDT, isOutput=False)
    bv = nc.declare_dram_parameter("bv", [1, 2048], RDT, isOutput=False)
    mask = nc.declare_dram_parameter("mask", [16, 128, 512], mybir.dt.bfloat16,
                                     isOutput=False)
    wot = nc.declare_dram_parameter("wot", [128, 4096], RDT, isOutput=False)
    onr = nc.declare_dram_parameter("onr", [1, 128], RDT, isOutput=False)
    idn = nc.declare_dram_parameter("idn", [128, 128], mybir.dt.bfloat16,
                                    isOutput=False)
    onc = nc.declare_dram_parameter("onc", [128, 128], RDT, isOutput=False)
    y = nc.declare_dram_parameter("y", [512, 256], FDT, isOutput=True)

    with TileContext(nc) as tc:
        with (
            tc.tile_pool(name="const", bufs=1) as cpool,
            tc.tile_pool(name="w4k", bufs=3) as wqpool,
            tc.tile_pool(name="v2k", bufs=6) as vpool,
            tc.tile_pool(name="qk4k", bufs=4) as qkpool,
            tc.tile_pool(name="work", bufs=2) as wpool,
            tc.tile_pool(name="ps_a", bufs=4, space="PSUM") as ps_a,
            tc.tile_pool(name="ps_o", bufs=1, space="PSUM") as ps_o,
            tc.tile_pool(name="ps_se", bufs=1, space="PSUM") as ps_se,
        ):
            def load(pool, name, src, shape, dt=FDT, tag=None):
                t = pool.tile(shape, dt, tag=tag or name, name=name)
                nc.sync.dma_start(out=t[:, :], in_=src)
                return t

            def mm(out, lhsT, rhs, **kw):
                nc.tensor.matmul(out, lhsT, rhs, **kw)

            xt_sb = [load(cpool, "xt0", xt0[:, :], [128, 512], dt=RDT),
                     load(cpool, "xt1", xt1[:, :], [128, 512], dt=RDT)]
            bqk_sb = load(cpool, "bqk", bqk[:, :], [128, 32])
            wqk_srcs = [wqk0, wqk1]
            wqk_sb = []
            for d in range(2):
                t = wqpool.tile([128, 4096], RDT, tag="w4k", name=f"wqk{d}")
                for ch in range(4):
                    nc.sync.dma_start(
                        out=t[:, ch * 1024:(ch + 1) * 1024],
                        in_=wqk_srcs[d][:, ch * 1024:(ch + 1) * 1024],
                    )
                wqk_sb.append(t)
            wv_sb = [load(vpool, "wv0", wv0[:, :], [128, 2048], dt=RDT, tag="v2k"),
                     load(vpool, "wv1", wv1[:, :], [128, 2048], dt=RDT, tag="v2k")]
            bv_sb = load(cpool, "bv", bv[:, :], [1, 2048], dt=RDT)

            ones_r = load(cpool, "ones_r", onr[:, :], [1, 128], dt=RDT)
            idn_sb = load(cpool, "idn", idn[:, :], [128, 128],
                          dt=mybir.dt.bfloat16)
            ones_c = load(cpool, "ones_c", onc[:, :], [128, 128], dt=RDT)

            # d-major Q^T/K^T, packed per d-chunk: cols = hl*1024 + u
            qt = [qkpool.tile([128, 4096], RDT, tag="qk4k", name=f"qt{c}")
                  for c in range(2)]
            kt = [qkpool.tile([128, 4096], RDT, tag="qk4k", name=f"kt{c}")
                  for c in range(2)]
            # V per head, u-major: [128 r, cb*256 + d]
            v_sb = [vpool.tile([128, 2048], RDT, tag="v2k", name=f"v{i}")
                    for i in range(HPC)]

            # ---- P1: Q^T/K^T projections, d-major (all 4 heads at once) ----
            for s in range(2):          # 0 = Q, 1 = K
                dst = qt if s == 0 else kt
                for cb in range(8):
                    for c in range(2):
                        ps = ps_a.tile([128, 512], FDT, tag="proj", name="proj")
                        for d in range(2):
                            mm(
                                ps[:, :],
                                wqk_sb[d][:, s * 2048 + cb * 256 + c * 128:
                                          s * 2048 + cb * 256 + c * 128 + 128],
                                xt_sb[d][:, :],
                                start=(d == 0), stop=(d == 1),
                            )
                        bi = s * 16 + cb * 2 + c
                        nc.vector.tensor_scalar_add(
                            out=dst[c].rearrange("p (h u) -> p h u", h=4)
                                [:, :, cb * 128:(cb + 1) * 128],
                            in0=ps.rearrange("p (h r) -> p h r", h=4)[:, :, :],
                            scalar1=bqk_sb[:, bi:bi + 1],
                        )

            # ---- P2: V projection per head (t-major == u-major here) ----
            for hl in range(HPC):
                for nn in range(4):
                    ps = ps_a.tile([128, 512], FDT, tag="proj", name="vproj")
                    for d in range(2):
                        mm(
                            ps[:, :],
                            xt_sb[d][:, hl * 128:hl * 128 + 128],
                            wv_sb[d][:, nn * 512:(nn + 1) * 512],
                            start=(d == 0), stop=False,
                        )
                    mm(
                        ps[:, :],
                        ones_r[:, :],
                        bv_sb[:, nn * 512:(nn + 1) * 512],
                        start=False, stop=True,
                    )
                    nc.vector.tensor_copy(
                        out=v_sb[hl][:, nn * 512:(nn + 1) * 512], in_=ps[:, :]
                    )

            # output of attention, normalized, packed: cols = hl*1024 + u
            oh = [wqpool.tile([128, 4096], RDT, tag="w4k", name=f"oh{c}")
                  for c in range(2)]
            wot_sb = load(wqpool, "wot", wot[:, :], [128, 4096], dt=RDT, tag="w4k")

            mask_sb = [load(cpool, f"mask{i}", mask[i], [128, 512],
                            mybir.dt.bfloat16) for i in range(16)]

            # ---- P3: attention per head ----
            for hl in range(HPC):
                for qj in range(2):
                    po = [ps_o.tile([128, 512], FDT, tag="po", name=f"po{c}",
                                    bufs=3) for c in range(2)]
                    se = ps_se.tile([128, 512], FDT, tag="se", name="se",
                                    bufs=1)
                    for ki in range(8):
                        sp = ps_a.tile([128, 512], FDT, tag="proj",
                                       name="score")
                        for c in range(2):
                            mm(
                                sp[:, :],
                                kt[c][:, hl * 1024 + ki * 128:
                                      hl * 1024 + ki * 128 + 128],
                                qt[c][:, hl * 1024 + qj * 512:
                                      hl * 1024 + qj * 512 + 512],
                                start=(c == 0), stop=(c == 1),
                            )
                        pt = wpool.tile([128, 512], RDT, tag="pt", name="pt",
                                        bufs=6)
                        nc.vector.tensor_add(
                            out=pt[:, :], in0=sp[:, :],
                            in1=mask_sb[ki * 2 + qj][:, :],
                        )
                        nc.scalar.activation(
                            pt[:, :], pt[:, :],
                            mybir.ActivationFunctionType.Exp
                        )
                        for c in range(2):
                            mm(
                                po[c][:, :],
                                v_sb[hl][:, ki * 256 + c * 128:
                                         ki * 256 + c * 128 + 128],
                                pt[:, :],
                                start=(ki == 0), stop=(ki == 7),
                            )
                        mm(
                            se[:, :], ones_c[:, :], pt[:, :],
                            start=(ki == 0), stop=(ki == 7),
                        )
                    rc = wpool.tile([128, 512], FDT, tag="rc", name="rc",
                                    bufs=3)
                    nc.vector.reciprocal(out=rc[:, :], in_=se[:, :])
                    for c in range(2):
                        nc.vector.tensor_mul(
                            out=oh[c][:, hl * 1024 + qj * 512:
                                      hl * 1024 + qj * 512 + 512],
                            in0=po[c][:, :], in1=rc[:, :],
                        )

            # ---- P4: output projection per head (y rows are per-head!) ----
            for hl in range(HPC):
                yp = ps_a.tile([128, 512], FDT, tag="proj", name="yproj")
                for cb in range(8):
                    for c in range(2):
                        j = 2 * cb + c
                        mm(
                            yp[:, 0:256],
                            oh[c][:, hl * 1024 + cb * 128:
                                  hl * 1024 + cb * 128 + 128],
                            wot_sb[:, j * 256:(j + 1) * 256],
                            start=(j == 0), stop=(j == 15),
                        )
                ys = wpool.tile([128, 256], FDT, tag="ys", name="ys")
                nc.vector.tensor_copy(out=ys[:, :], in_=yp[:, 0:256])
                nc.sync.dma_start(
                    out=y[hl * 128:(hl + 1) * 128, :], in_=ys[:, :]
                )
    nc.finalize()
    return nc


def _prep_inputs(x, w_attn, b_attn, w_out):
    # causal mask on pseudo-positions s', in permuted order u = cb*128 + r
    rk = np.arange(128)
    rq = np.arange(512)
    masks = np.empty((16, 128, 512), dtype=ml_dtypes.bfloat16)
    for ki in range(8):
        for qj in range(2):
            spk = rk * 8 + ki                                # s' of key rows
            spq = (rq % 128) * 8 + (qj * 4 + rq // 128)      # s' of query cols
            masks[ki * 2 + qj] = np.where(
                spk[:, None] <= spq[None, :], 0.0, NEG
            ).astype(ml_dtypes.bfloat16)

    wqk = np.ascontiguousarray(
        np.concatenate([w_attn[0:2048] / 16.0, w_attn[2048:4096]]).T
    )  # (256, 4096)
    wvt = np.ascontiguousarray(w_attn[4096:6144].T)  # (256, 2048)
    bqk_arr = np.ascontiguousarray(
        np.concatenate([b_attn[0:2048] / 16.0, b_attn[2048:4096]])
        .reshape(32, 128).T
    )  # (128, 32)
    bv_arr = b_attn[4096:6144].reshape(1, 2048).astype(np.float32)
    wot_arr = np.ascontiguousarray(
        w_out.T.reshape(16, 128, 256).transpose(1, 0, 2).reshape(128, 4096)
    )

    in_maps = []
    for c in range(NCORES):
        b, g = divmod(c, 2)
        xt = np.ascontiguousarray(x[b, 512 * g:512 * (g + 1)].T)  # (256, 512)
        in_maps.append({
            "xt0": np.ascontiguousarray(xt[:128]),
            "xt1": np.ascontiguousarray(xt[128:]),
            "wqk0": np.ascontiguousarray(wqk[:128]),
            "wqk1": np.ascontiguousarray(wqk[128:]),
            "wv0": np.ascontiguousarray(wvt[:128]),
            "wv1": np.ascontiguousarray(wvt[128:]),
            "bqk": bqk_arr.astype(np.float32),
            "bv": bv_arr,
            "mask": masks,
            "wot": wot_arr.astype(np.float32),
            "onr": np.ones((1, 128), np.float32),
            "idn": np.eye(128, dtype=ml_dtypes.bfloat16),
            "onc": np.ones((128, 128), np.float32),
        })
    return in_maps


def kernel(x, w_attn, b_attn, w_out, b_out):
    x = np.asarray(x, dtype=np.float32)
    w_attn = np.asarray(w_attn, dtype=np.float32)
    b_attn = np.asarray(b_attn, dtype=np.float32)
    w_out = np.asarray(w_out, dtype=np.float32)
    b_out = np.asarray(b_out, dtype=np.float32)

    if "nc" not in _cache:
        _cache["nc"] = _build()
    nc = _cache["nc"]

    in_maps = _prep_inputs(x, w_attn, b_attn, w_out)
    res = run_bass_kernel_spmd(nc, in_maps, list(range(NCORES))).results

    out = np.empty((BS, SQL, EDIM), dtype=np.float32)
    for c in range(NCORES):
        b, g = divmod(c, 2)
        out[b, 512 * g:512 * (g + 1)] = res[c]["y"]
    out += b_out
    return out



# revision 3
# speedup vs baseline: 1.1981x; 1.1981x over previous
"""Masked multi-head attention on 8 NeuronCores (faithful torch raw-view semantics).

The reference reshapes (bs, sql, nh*edim) -> (bs, nh, sql, edim) as a RAW VIEW:
head h's length-1024 pseudo-sequence comes from x rows 128h..128h+127, each row
contributing 8 pseudo-positions s' = 8r + cb (cb = 256-col block of the
projection). Work splits into 32 independent (batch, head) pairs -> 4 per core.

This version keeps Q^T/K^T/V^T in NATURAL s' order (the PSUM->SBUF bias-copies
scatter columns via strided APs at zero extra cost), which makes the causal
structure block-aligned: score/PV/denominator matmuls only cover the unmasked
column suffix [p, 512) per key block (saves ~44% of attention PE columns), and
the mask shrinks to one shared [128,128] triangle added only on diagonal
blocks. V is transposed per 128-key natural block on the PE (bf16, via
identity) so P@V contracts keys on partitions; pt/V are bf16 (full PE rate at
any width). Softmax denominator via ones-matmul into a per-suffix PSUM
accumulation; normalization multiplies with a reciprocal row and scatters
straight back to the PERMUTED oh layout that the output projection wants.
"""

import sys

sys.path.insert(0, "/opt/trn_rl_repo")

import ml_dtypes
import numpy as np

from concourse import bacc, mybir
from concourse.tile import TileContext
from concourse.bass_utils import run_bass_kernel_spmd

EDIM = 256
BS = 4
SQL = 1024
HPC = 4           # heads per core
NCORES = 8
FDT = mybir.dt.float32
RDT = mybir.dt.float32r    # matmul-feeding storage: full-rate PE, rounded fp32
BDT = mybir.dt.bfloat16
NEG = -1.0e30

_cache = {}


def _build():
    nc = bacc.Bacc(dynamic_dma_scratch_size=512)

    xt0 = nc.declare_dram_parameter("xt0", [128, 512], RDT, isOutput=False)
    xt1 = nc.declare_dram_parameter("xt1", [128, 512], RDT, isOutput=False)
    # col sections: [V, Q/16, K], each 2048 wide; within a section: cb*256+c*128
    wqkv0 = nc.declare_dram_parameter("wqkv0", [128, 6144], RDT, isOutput=False)
    wqkv1 = nc.declare_dram_parameter("wqkv1", [128, 6144], RDT, isOutput=False)
    bqkv = nc.declare_dram_parameter("bqkv", [128, 48], FDT, isOutput=False)
    wot = nc.declare_dram_parameter("wot", [128, 4096], RDT, isOutput=False)
    tri = nc.declare_dram_parameter("tri", [128, 128], BDT, isOutput=False)
    idn = nc.declare_dram_parameter("idn", [128, 128], BDT, isOutput=False)
    onc = nc.declare_dram_parameter("onc", [128, 128], BDT, isOutput=False)
    y = nc.declare_dram_parameter("y", [512, 256], FDT, isOutput=True)

    EXP = mybir.ActivationFunctionType.Exp

    with TileContext(nc) as tc:
        with (
            tc.tile_pool(name="const", bufs=1) as cpool,
            tc.tile_pool(name="big", bufs=3) as bigpool,
            tc.tile_pool(name="qk", bufs=1) as qkpool,
            tc.tile_pool(name="vkp", bufs=4) as vkpool,
            tc.tile_pool(name="ptp", bufs=4) as ptpool,
            tc.tile_pool(name="work", bufs=2) as wpool,
            tc.tile_pool(name="ps_a", bufs=4, space="PSUM") as ps_a,
            tc.tile_pool(name="ps_po", bufs=2, space="PSUM") as ps_po,
            tc.tile_pool(name="ps_se", bufs=2, space="PSUM") as ps_se,
        ):
            def load(pool, name, src, shape, dt=FDT, tag=None):
                t = pool.tile(shape, dt, tag=tag or name, name=name)
                nc.sync.dma_start(out=t[:, :], in_=src)
                return t

            def mm(out, lhsT, rhs, **kw):
                nc.tensor.matmul(out, lhsT, rhs, **kw)

            xt_sb = [load(cpool, "xt0", xt0[:, :], [128, 512], dt=RDT),
                     load(cpool, "xt1", xt1[:, :], [128, 512], dt=RDT)]
            bqkv_sb = load(cpool, "bqkv", bqkv[:, :], [128, 48])
            tri_sb = load(cpool, "tri", tri[:, :], [128, 128], dt=BDT)
            idn_sb = load(cpool, "idn", idn[:, :], [128, 128], dt=BDT)
            onc_sb = load(cpool, "onc", onc[:, :], [128, 128], dt=BDT)

            wqkv_srcs = [wqkv0, wqkv1]
            wqkv_sb = []
            for d in range(2):
                t = bigpool.tile([128, 6144], RDT, tag="big", name=f"wqkv{d}")
                for ch in range(6):
                    nc.sync.dma_start(
                        out=t[:, ch * 1024:(ch + 1) * 1024],
                        in_=wqkv_srcs[d][:, ch * 1024:(ch + 1) * 1024],
                    )
                wqkv_sb.append(t)
            wot_sb = bigpool.tile([128, 6144], RDT, tag="big", name="wot")
            for ch in range(4):
                nc.sync.dma_start(
                    out=wot_sb[:, ch * 1024:(ch + 1) * 1024],
                    in_=wot[:, ch * 1024:(ch + 1) * 1024],
                )

            # natural-order projections: col = h*1024 + 8r + cb
            qt = [qkpool.tile([128, 4096], RDT, tag=f"qt{c}", name=f"qt{c}")
                  for c in range(2)]
            kt = [qkpool.tile([128, 4096], RDT, tag=f"kt{c}", name=f"kt{c}")
                  for c in range(2)]
            vt = [qkpool.tile([128, 4096], BDT, tag=f"vt{c}", name=f"vt{c}")
                  for c in range(2)]

            # ---- P1: V^T, Q^T, K^T projections (V first: feeds transposes) ----
            for s in range(3):          # 0 = V, 1 = Q, 2 = K
                dst = (vt, qt, kt)[s]
                for cb in range(8):
                    for c in range(2):
                        ti = s * 16 + cb * 2 + c
                        ps = ps_a.tile([128, 512], FDT, tag="ps", name="proj")
                        for d in range(2):
                            mm(
                                ps[:, :],
                                wqkv_sb[d][:, s * 2048 + cb * 256 + c * 128:
                                           s * 2048 + cb * 256 + c * 128 + 128],
                                xt_sb[d][:, :],
                                start=(d == 0), stop=(d == 1),
                            )
                        out_ap = dst[c].rearrange(
                            "p (h r e) -> p h r e", h=4, r=128)[:, :, :, cb]
                        in_ap = ps.rearrange("p (h r) -> p h r", h=4)[:, :, :]
                        if ti % 2 == 0:
                            nc.scalar.add(out_ap, in_ap,
                                          add=bqkv_sb[:, ti:ti + 1])
                        else:
                            nc.vector.tensor_scalar_add(
                                out=out_ap, in0=in_ap,
                                scalar1=bqkv_sb[:, ti:ti + 1],
                            )

            # normalized attention out, PERMUTED cols (= hl*1024 + cb*128 + r)
            oh = [bigpool.tile([128, 6144], RDT, tag="big", name=f"oh{c}")
                  for c in range(2)]

            for hl in range(HPC):
                # ---- V_k: natural 128-key blocks transposed to [key, d] ----
                vk = vkpool.tile([128, 2048], BDT, tag="vk", name=f"vk{hl}")
                for c in range(2):
                    tv = ps_a.tile([128, 512], FDT, tag="ps", name="tv")
                    tvb = tv[:, :].bitcast(BDT)
                    for k in range(8):
                        nc.tensor.transpose(
                            tvb[:, k * 128:(k + 1) * 128],
                            vt[c][:, hl * 1024 + k * 128:
                                  hl * 1024 + k * 128 + 128],
                            idn_sb[:, :],
                        )
                    nc.vector.tensor_copy(
                        out=vk[:, c * 1024:(c + 1) * 1024], in_=tvb)

                # ---- attention, natural order, causal-suffix matmuls ----
                for qj in range(2):
                    kmax = 4 * qj + 3
                    po = [ps_po.tile([128, 512], FDT, tag="po", name=f"po{c}")
                          for c in range(2)]
                    se = ps_se.tile([128, 512], FDT, tag="se", name="se")
                    for k in range(kmax + 1):
                        p_t = max(0, 128 * (k - 4 * qj))
                        p_m = min(p_t, 256)   # fp32r needs >=256 moving cols
                        sp = ps_a.tile([128, 512], FDT, tag="ps", name="score")
                        for c in range(2):
                            mm(
                                sp[:, p_m:512],
                                kt[c][:, hl * 1024 + k * 128:
                                      hl * 1024 + k * 128 + 128],
                                qt[c][:, hl * 1024 + 512 * qj + p_m:
                                      hl * 1024 + 512 * qj + 512],
                                start=(c == 0), stop=(c == 1),
                            )
                        if k >= 4 * qj:
                            nc.vector.tensor_add(
                                out=sp[:, p_t:p_t + 128],
                                in0=sp[:, p_t:p_t + 128],
                                in1=tri_sb[:, :],
                            )
                        pt = ptpool.tile([128, 512], BDT, tag="pt", name="pt")
                        nc.scalar.activation(pt[:, p_t:512], sp[:, p_t:512],
                                             EXP)
                        for c in range(2):
                            mm(
                                po[c][:, p_t:512],
                                vk[:, c * 1024 + k * 128:
                                   c * 1024 + k * 128 + 128],
                                pt[:, p_t:512],
                                start=(k == 0), stop=(k == kmax),
                            )
                        mm(
                            se[:, p_t:512], onc_sb[:, :], pt[:, p_t:512],
                            start=(k == 0), stop=(k == kmax),
                        )
                    rc = wpool.tile([128, 512], FDT, tag="rc", name="rc")
                    nc.vector.reciprocal(out=rc[:, :], in_=se[:, :])
                    for c in range(2):
                        out_ap = oh[c][:, 0:4096].rearrange(
                            "p (h cb r) -> p h cb r", h=4, cb=8
                        )[:, hl, :, 64 * qj:64 * qj + 64]
                        nc.vector.tensor_mul(
                            out=out_ap,
                            in0=po[c][:, :].rearrange("p (th cb) -> p cb th",
                                                      cb=8),
                            in1=rc[:, :].rearrange("p (th cb) -> p cb th",
                                                   cb=8),
                        )

                # ---- output projection for this head (y rows are per-head) --
                yp = ps_se.tile([128, 512], FDT, tag="se", name="yp")
                for cb in range(8):
                    for c in range(2):
                        j = 2 * cb + c
                        mm(
                            yp[:, 0:256],
                            oh[c][:, hl * 1024 + cb * 128:
                                  hl * 1024 + cb * 128 + 128],
                            wot_sb[:, j * 256:(j + 1) * 256],
                            start=(j == 0), stop=(j == 15),
                        )
                ys = wpool.tile([128, 256], FDT, tag="ys", name="ys")
                nc.vector.tensor_copy(out=ys[:, :], in_=yp[:, 0:256])
                nc.sync.dma_start(
                    out=y[hl * 128:(hl + 1) * 128, :], in_=ys[:, :]
                )
    nc.finalize()
    return nc


def _prep_inputs(x, w_attn, b_attn, w_out):
    # shared diagonal-block causal mask in natural order: masked iff key > query
    r = np.arange(128)
    tri_arr = np.where(r[:, None] <= r[None, :], 0.0, NEG).astype(
        ml_dtypes.bfloat16)

    wqkv = np.ascontiguousarray(
        np.concatenate([w_attn[4096:6144], w_attn[0:2048] / 16.0,
                        w_attn[2048:4096]]).T
    )  # (256, 6144), sections [V, Q/16, K]
    bqkv_arr = np.ascontiguousarray(
        np.concatenate([b_attn[4096:6144], b_attn[0:2048] / 16.0,
                        b_attn[2048:4096]]).reshape(48, 128).T
    ).astype(np.float32)  # (128, 48)
    wot_arr = np.ascontiguousarray(
        w_out.T.reshape(16, 128, 256).transpose(1, 0, 2).reshape(128, 4096)
    ).astype(np.float32)

    in_maps = []
    for c in range(NCORES):
        b, g = divmod(c, 2)
        xt = np.ascontiguousarray(x[b, 512 * g:512 * (g + 1)].T)  # (256, 512)
        in_maps.append({
            "xt0": np.ascontiguousarray(xt[:128]),
            "xt1": np.ascontiguousarray(xt[128:]),
            "wqkv0": np.ascontiguousarray(wqkv[:128]),
            "wqkv1": np.ascontiguousarray(wqkv[128:]),
            "bqkv": bqkv_arr,
            "wot": wot_arr,
            "tri": tri_arr,
            "idn": np.eye(128, dtype=ml_dtypes.bfloat16),
            "onc": np.ones((128, 128), ml_dtypes.bfloat16),
        })
    return in_maps


def kernel(x, w_attn, b_attn, w_out, b_out):
    x = np.asarray(x, dtype=np.float32)
    w_attn = np.asarray(w_attn, dtype=np.float32)
    b_attn = np.asarray(b_attn, dtype=np.float32)
    w_out = np.asarray(w_out, dtype=np.float32)
    b_out = np.asarray(b_out, dtype=np.float32)

    if "nc" not in _cache:
        _cache["nc"] = _build()
    nc = _cache["nc"]

    in_maps = _prep_inputs(x, w_attn, b_attn, w_out)
    res = run_bass_kernel_spmd(nc, in_maps, list(range(NCORES))).results

    out = np.empty((BS, SQL, EDIM), dtype=np.float32)
    for c in range(NCORES):
        b, g = divmod(c, 2)
        out[b, 512 * g:512 * (g + 1)] = res[c]["y"]
    out += b_out
    return out


# revision 4
# speedup vs baseline: 1.3363x; 1.1154x over previous
"""Masked multi-head attention on 8 NeuronCores (faithful torch raw-view semantics).

The reference reshapes (bs, sql, nh*edim) -> (bs, nh, sql, edim) as a RAW VIEW:
head h's length-1024 pseudo-sequence comes from x rows 128h..128h+127, each row
contributing 8 pseudo-positions s' = 8r + cb (cb = 256-col block of the
projection). Work splits into 32 independent (batch, head) pairs -> 4 per core.

Q^T/K^T/V^T are kept in NATURAL s' order (the PSUM->SBUF bias-copies scatter
columns via strided APs at zero extra cost), which makes the causal structure
block-aligned: score/PV/denominator matmuls only cover the unmasked column
suffix [p, 512) per key block (saves ~44% of attention PE columns), and the
mask shrinks to one shared [128,128] triangle added only on diagonal blocks.
V is transposed per 128-key natural block on the PE (bf16, via identity) so
P@V contracts keys on partitions; pt/V/oh/wot are bf16 (full PE rate at any
width, half DMA). Softmax denominator via ones-matmul accumulated per column
suffix; normalization multiplies by a reciprocal row and scatters straight
back to the PERMUTED oh layout the output projection wants. Q weights/bias
pre-scaled by 1/16. V projection runs fully in bf16 (it feeds bf16 V anyway),
Q/K projections and scores stay fp32r.
"""

import sys

sys.path.insert(0, "/opt/trn_rl_repo")

import ml_dtypes
import numpy as np

from concourse import bacc, mybir
from concourse.tile import TileContext
from concourse.bass_utils import run_bass_kernel_spmd

EDIM = 256
BS = 4
SQL = 1024
HPC = 4           # heads per core
NCORES = 8
FDT = mybir.dt.float32
RDT = mybir.dt.float32r    # matmul-feeding storage: full-rate PE, rounded fp32
BDT = mybir.dt.bfloat16
NEG = -1.0e30

_cache = {}


def _build():
    nc = bacc.Bacc(dynamic_dma_scratch_size=512)

    xt0 = nc.declare_dram_parameter("xt0", [128, 512], RDT, isOutput=False)
    xt1 = nc.declare_dram_parameter("xt1", [128, 512], RDT, isOutput=False)
    xb0 = nc.declare_dram_parameter("xb0", [128, 512], BDT, isOutput=False)
    xb1 = nc.declare_dram_parameter("xb1", [128, 512], BDT, isOutput=False)
    # V weights (bf16): cols cb*256 + c*128 + d'
    wv0 = nc.declare_dram_parameter("wv0", [128, 2048], BDT, isOutput=False)
    wv1 = nc.declare_dram_parameter("wv1", [128, 2048], BDT, isOutput=False)
    # Q/16 and K weights (f32r): sections [Q/16, K], each 2048 wide
    wqk0 = nc.declare_dram_parameter("wqk0", [128, 4096], RDT, isOutput=False)
    wqk1 = nc.declare_dram_parameter("wqk1", [128, 4096], RDT, isOutput=False)
    bqkv = nc.declare_dram_parameter("bqkv", [128, 48], FDT, isOutput=False)
    wot = nc.declare_dram_parameter("wot", [128, 4096], BDT, isOutput=False)
    tri = nc.declare_dram_parameter("tri", [128, 128], BDT, isOutput=False)
    idn = nc.declare_dram_parameter("idn", [128, 128], BDT, isOutput=False)
    onc = nc.declare_dram_parameter("onc", [128, 128], BDT, isOutput=False)
    y = nc.declare_dram_parameter("y", [512, 256], FDT, isOutput=True)

    EXP = mybir.ActivationFunctionType.Exp

    with TileContext(nc) as tc:
        with (
            tc.tile_pool(name="const", bufs=1) as cpool,
            tc.tile_pool(name="wqkp", bufs=2) as wqkpool,
            tc.tile_pool(name="wvp", bufs=2) as wvpool,
            tc.tile_pool(name="qk", bufs=1) as qkpool,
            tc.tile_pool(name="ohp", bufs=2) as ohpool,
            tc.tile_pool(name="vkp", bufs=4) as vkpool,
            tc.tile_pool(name="ptp", bufs=4) as ptpool,
            tc.tile_pool(name="work", bufs=2) as wpool,
            tc.tile_pool(name="ps_a", bufs=4, space="PSUM") as ps_a,
            tc.tile_pool(name="ps_po", bufs=2, space="PSUM") as ps_po,
            tc.tile_pool(name="ps_se", bufs=2, space="PSUM") as ps_se,
        ):
            def load(pool, name, src, shape, dt=FDT, tag=None):
                t = pool.tile(shape, dt, tag=tag or name, name=name)
                nc.sync.dma_start(out=t[:, :], in_=src)
                return t

            def mm(out, lhsT, rhs, **kw):
                nc.tensor.matmul(out, lhsT, rhs, **kw)

            # DMA order = need order: x first, then V weights (bf16, consumed
            # first), Q/K weight chunks interleaved d0/d1, then the rest.
            xb_sb = [load(cpool, "xb0", xb0[:, :], [128, 512], dt=BDT),
                     load(cpool, "xb1", xb1[:, :], [128, 512], dt=BDT)]
            wv_sb = [load(cpool, "wv0", wv0[:, :], [128, 2048], dt=BDT),
                     load(cpool, "wv1", wv1[:, :], [128, 2048], dt=BDT)]
            bqkv_sb = load(cpool, "bqkv", bqkv[:, :], [128, 48])
            xt_sb = [load(cpool, "xt0", xt0[:, :], [128, 512], dt=RDT),
                     load(cpool, "xt1", xt1[:, :], [128, 512], dt=RDT)]
            wqk_srcs = [wqk0, wqk1]
            wqk_sb = [wqkpool.tile([128, 4096], RDT, tag="wqk", name=f"wqk{d}")
                      for d in range(2)]
            for ch in range(4):
                for d in range(2):
                    nc.sync.dma_start(
                        out=wqk_sb[d][:, ch * 1024:(ch + 1) * 1024],
                        in_=wqk_srcs[d][:, ch * 1024:(ch + 1) * 1024],
                    )
            idn_sb = load(cpool, "idn", idn[:, :], [128, 128], dt=BDT)
            tri_sb = load(cpool, "tri", tri[:, :], [128, 128], dt=BDT)
            onc_sb = load(cpool, "onc", onc[:, :], [128, 128], dt=BDT)
            wot_sb = cpool.tile([128, 4096], BDT, tag="wot", name="wot")
            for ch in range(2):
                nc.sync.dma_start(
                    out=wot_sb[:, ch * 2048:(ch + 1) * 2048],
                    in_=wot[:, ch * 2048:(ch + 1) * 2048],
                )

            # natural-order projections: col = h*1024 + 8r + cb
            qt = [qkpool.tile([128, 4096], RDT, tag=f"qt{c}", name=f"qt{c}")
                  for c in range(2)]
            kt = [qkpool.tile([128, 4096], RDT, tag=f"kt{c}", name=f"kt{c}")
                  for c in range(2)]
            vt = [qkpool.tile([128, 4096], BDT, tag=f"vt{c}", name=f"vt{c}")
                  for c in range(2)]

            # ---- P1: V^T, Q^T, K^T projections (V first: feeds transposes) ----
            for s in range(3):          # 0 = V, 1 = Q, 2 = K
                dst = (vt, qt, kt)[s]
                for cb in range(8):
                    for c in range(2):
                        ti = s * 16 + cb * 2 + c
                        ps = ps_a.tile([128, 512], FDT, tag="ps", name="proj")
                        for d in range(2):
                            if s == 0:
                                w_ap = wv_sb[d][:, cb * 256 + c * 128:
                                                cb * 256 + c * 128 + 128]
                                x_ap = xb_sb[d][:, :]
                            else:
                                w_ap = wqk_sb[d][:, (s - 1) * 2048 + cb * 256
                                                 + c * 128:
                                                 (s - 1) * 2048 + cb * 256
                                                 + c * 128 + 128]
                                x_ap = xt_sb[d][:, :]
                            mm(ps[:, :], w_ap, x_ap,
                               start=(d == 0), stop=(d == 1))
                        out_ap = dst[c].rearrange(
                            "p (h r e) -> p h r e", h=4, r=128)[:, :, :, cb]
                        in_ap = ps.rearrange("p (h r) -> p h r", h=4)[:, :, :]
                        if ti % 2 == 0:
                            nc.scalar.add(out_ap, in_ap,
                                          add=bqkv_sb[:, ti:ti + 1])
                        else:
                            nc.vector.tensor_scalar_add(
                                out=out_ap, in0=in_ap,
                                scalar1=bqkv_sb[:, ti:ti + 1],
                            )

            # normalized attention out, PERMUTED cols (= hl*1024 + cb*128 + r)
            oh = [ohpool.tile([128, 4096], BDT, tag="oh", name=f"oh{c}")
                  for c in range(2)]

            for hl in range(HPC):
                # ---- V_k: natural 128-key blocks transposed to [key, d] ----
                vk = vkpool.tile([128, 2048], BDT, tag="vk", name=f"vk{hl}")
                for c in range(2):
                    tv = ps_a.tile([128, 512], FDT, tag="ps", name="tv")
                    tvb = tv[:, :].bitcast(BDT)
                    for k in range(8):
                        nc.tensor.transpose(
                            tvb[:, k * 128:(k + 1) * 128],
                            vt[c][:, hl * 1024 + k * 128:
                                  hl * 1024 + k * 128 + 128],
                            idn_sb[:, :],
                        )
                    nc.vector.tensor_copy(
                        out=vk[:, c * 1024:(c + 1) * 1024], in_=tvb)

                # ---- attention, natural order, causal-suffix matmuls ----
                for qj in range(2):
                    kmax = 4 * qj + 3
                    po = [ps_po.tile([128, 512], FDT, tag="po", name=f"po{c}")
                          for c in range(2)]
                    se = ps_se.tile([128, 512], FDT, tag="se", name="se")
                    for k in range(kmax + 1):
                        p_t = max(0, 128 * (k - 4 * qj))
                        p_m = min(p_t, 256)   # fp32r needs >=256 moving cols
                        sp = ps_a.tile([128, 512], FDT, tag="ps", name="score")
                        for c in range(2):
                            mm(
                                sp[:, p_m:512],
                                kt[c][:, hl * 1024 + k * 128:
                                      hl * 1024 + k * 128 + 128],
                                qt[c][:, hl * 1024 + 512 * qj + p_m:
                                      hl * 1024 + 512 * qj + 512],
                                start=(c == 0), stop=(c == 1),
                            )
                        if k >= 4 * qj:
                            nc.vector.tensor_add(
                                out=sp[:, p_t:p_t + 128],
                                in0=sp[:, p_t:p_t + 128],
                                in1=tri_sb[:, :],
                            )
                        pt = ptpool.tile([128, 512], BDT, tag="pt", name="pt")
                        nc.scalar.activation(pt[:, p_t:512], sp[:, p_t:512],
                                             EXP)
                        for c in range(2):
                            mm(
                                po[c][:, p_t:512],
                                vk[:, c * 1024 + k * 128:
                                   c * 1024 + k * 128 + 128],
                                pt[:, p_t:512],
                                start=(k == 0), stop=(k == kmax),
                            )
                        mm(
                            se[:, p_t:512], onc_sb[:, :], pt[:, p_t:512],
                            start=(k == 0), stop=(k == kmax),
                        )
                    rc = wpool.tile([128, 512], FDT, tag="rc", name="rc")
                    nc.vector.reciprocal(out=rc[:, :], in_=se[:, :])
                    for c in range(2):
                        out_ap = oh[c].rearrange(
                            "p (h cb r) -> p h cb r", h=4, cb=8
                        )[:, hl, :, 64 * qj:64 * qj + 64]
                        nc.vector.tensor_mul(
                            out=out_ap,
                            in0=po[c][:, :].rearrange("p (th cb) -> p cb th",
                                                      cb=8),
                            in1=rc[:, :].rearrange("p (th cb) -> p cb th",
                                                   cb=8),
                        )

                # ---- output projection for this head (y rows are per-head) --
                yp = ps_se.tile([128, 512], FDT, tag="se", name="yp")
                for cb in range(8):
                    for c in range(2):
                        j = 2 * cb + c
                        mm(
                            yp[:, 0:256],
                            oh[c][:, hl * 1024 + cb * 128:
                                  hl * 1024 + cb * 128 + 128],
                            wot_sb[:, j * 256:(j + 1) * 256],
                            start=(j == 0), stop=(j == 15),
                        )
                ys = wpool.tile([128, 256], FDT, tag="ys", name="ys")
                nc.scalar.copy(out=ys[:, :], in_=yp[:, 0:256])
                nc.sync.dma_start(
                    out=y[hl * 128:(hl + 1) * 128, :], in_=ys[:, :]
                )
    nc.finalize()
    return nc


def _prep_inputs(x, w_attn, b_attn, w_out):
    # shared diagonal-block causal mask in natural order: masked iff key > query
    r = np.arange(128)
    tri_arr = np.where(r[:, None] <= r[None, :], 0.0, NEG).astype(
        ml_dtypes.bfloat16)

    wv = np.ascontiguousarray(w_attn[4096:6144].T).astype(
        ml_dtypes.bfloat16)      # (256, 2048)
    wqk = np.ascontiguousarray(
        np.concatenate([w_attn[0:2048] / 16.0, w_attn[2048:4096]]).T
    )  # (256, 4096), sections [Q/16, K]
    bqkv_arr = np.ascontiguousarray(
        np.concatenate([b_attn[4096:6144], b_attn[0:2048] / 16.0,
                        b_attn[2048:4096]]).reshape(48, 128).T
    ).astype(np.float32)  # (128, 48), tile order [V, Q, K]
    wot_arr = np.ascontiguousarray(
        w_out.T.reshape(16, 128, 256).transpose(1, 0, 2).reshape(128, 4096)
    ).astype(ml_dtypes.bfloat16)

    in_maps = []
    for c in range(NCORES):
        b, g = divmod(c, 2)
        xt = np.ascontiguousarray(x[b, 512 * g:512 * (g + 1)].T)  # (256, 512)
        in_maps.append({
            "xt0": np.ascontiguousarray(xt[:128]),
            "xt1": np.ascontiguousarray(xt[128:]),
            "xb0": np.ascontiguousarray(xt[:128]).astype(ml_dtypes.bfloat16),
            "xb1": np.ascontiguousarray(xt[128:]).astype(ml_dtypes.bfloat16),
            "wv0": np.ascontiguousarray(wv[:128]),
            "wv1": np.ascontiguousarray(wv[128:]),
            "wqk0": np.ascontiguousarray(wqk[:128]),
            "wqk1": np.ascontiguousarray(wqk[128:]),
            "bqkv": bqkv_arr,
            "wot": wot_arr,
            "tri": tri_arr,
            "idn": np.eye(128, dtype=ml_dtypes.bfloat16),
            "onc": np.ones((128, 128), ml_dtypes.bfloat16),
        })
    return in_maps


def kernel(x, w_attn, b_attn, w_out, b_out):
    x = np.asarray(x, dtype=np.float32)
    w_attn = np.asarray(w_attn, dtype=np.float32)
    b_attn = np.asarray(b_attn, dtype=np.float32)
    w_out = np.asarray(w_out, dtype=np.float32)
    b_out = np.asarray(b_out, dtype=np.float32)

    if "nc" not in _cache:
        _cache["nc"] = _build()
    nc = _cache["nc"]

    in_maps = _prep_inputs(x, w_attn, b_attn, w_out)
    res = run_bass_kernel_spmd(nc, in_maps, list(range(NCORES))).results

    out = np.empty((BS, SQL, EDIM), dtype=np.float32)
    for c in range(NCORES):
        b, g = divmod(c, 2)
        out[b, 512 * g:512 * (g + 1)] = res[c]["y"]
    out += b_out
    return out
